# revision 45
# baseline (speedup 1.0000x reference)
"""Trainium2 Bass kernel for nn_ColbertAdapter (ColBERT late-interaction adapter).

v2 strategy (8 NeuronCores, single SPMD launch, contexts sharded 256/core):
  - LN (stats on VectorE, normalize on ScalarE via Identity w/ AP scale+bias)
    -> DMA-XBAR transposes (no PE transposes, no evac copies).
  - Per head-pair jt: q/k projections for that jt only, then scores with the
    two heads' matmuls interleaved so the K=64 row-tiles (partitions 0-63 /
    64-127) run concurrently on the PE (auto tile_position).
  - PSUM drain split: head 2jt exp-first (ScalarE exp per u-tile, bf16 max
    tree on VectorE at 2x), head 2jt+1 max-first (ScalarE seed copy, VectorE
    fp32 max chain, one exp at the end). Balances ACT and DVE.
  - attn@v with ones-augmented v (softmax denominator from the same matmul),
    bf16 ReduceScatter per pair overlapped with the next pair's compute; a
    tiny dummy ReduceScatter at kernel start absorbs CC warm-up/skew.
  - Softmax uses a fixed zero max-offset (logit bound checked on host).
  - Per-pair tail: readback + denominator broadcast + divide as each RS
    lands; final wo/LN4/wp only after the last pair.
"""

import os
import sys

try:
    import concourse  # noqa: F401
except ImportError:
    for p in ("/opt/trn_rl_repo", "/root/.axon_site/_ro/trn_rl_repo"):
        if os.path.isdir(p):
            sys.path.insert(0, p)
            break

import numpy as np
import ml_dtypes

import concourse.bass as bass
import concourse.mybir as mybir
from concourse import tile, bacc, bass_utils
from concourse.alu_op_type import AluOpType

BF16 = mybir.dt.bfloat16
F32 = mybir.dt.float32
Ident = mybir.ActivationFunctionType.Identity
Exp = mybir.ActivationFunctionType.Exp
Sqrt = mybir.ActivationFunctionType.Sqrt

NCORES = 8
B, T, C, U, D, P = 4, 256, 2048, 4, 512, 512
H = 8
DK = D // H
BT = B * T              # 1024 query tokens
CS = C // NCORES        # 256 contexts per core
CUS = CS * U            # 1024 key rows per core
TSH = BT // NCORES      # 128 tokens per core in the output shard
EPS = 1e-5

_CACHE = {}


def build_nc():
    nc = bacc.Bacc("TRN2", target_bir_lowering=False, debug=False,
                   num_devices=NCORES)

    # ---- DRAM I/O ----
    x_d = nc.dram_tensor("x", [BT, D], BF16, kind="ExternalInput").ap()
    kin_d = nc.dram_tensor("kin", [CUS, D], BF16, kind="ExternalInput").ap()
    vin_d = nc.dram_tensor("vin", [CS, D], BF16, kind="ExternalInput").ap()
    w_d = {
        n: nc.dram_tensor(n, [D, D], BF16, kind="ExternalInput").ap()
        for n in ("wq", "wk", "wv", "wo", "wp")
    }
    bq_d = nc.dram_tensor("bq", [D], F32, kind="ExternalInput").ap()
    bk_d = nc.dram_tensor("bk", [D], F32, kind="ExternalInput").ap()
    bv_d = nc.dram_tensor("bv", [D], F32, kind="ExternalInput").ap()
    bo_d = nc.dram_tensor("bo", [D], BF16, kind="ExternalInput").ap()
    bp_d = nc.dram_tensor("bp", [D], BF16, kind="ExternalInput").ap()
    ind_d = nc.dram_tensor("ind", [2, 128], BF16, kind="ExternalInput").ap()
    eye_d = nc.dram_tensor("eye", [128, 128], BF16, kind="ExternalInput").ap()
    y_d = nc.dram_tensor("y", [TSH, P], F32, kind="ExternalOutput").ap()

    with tile.TileContext(nc) as tc:
        from contextlib import ExitStack
        ctx = ExitStack()
        with ctx:
            persist = ctx.enter_context(tc.tile_pool(name="persist", bufs=1))
            small = ctx.enter_context(tc.tile_pool(name="small", bufs=8))
            lnin = ctx.enter_context(tc.tile_pool(name="lnin", bufs=9))
            lnout = ctx.enter_context(tc.tile_pool(name="lnout", bufs=1))
            epool = ctx.enter_context(tc.tile_pool(name="epool", bufs=4))
            mpool = ctx.enter_context(tc.tile_pool(name="mpool", bufs=4))
            pmpool = ctx.enter_context(tc.tile_pool(name="pmpool", bufs=6))
            o65p = ctx.enter_context(tc.tile_pool(name="o65", bufs=5))
            pscore = ctx.enter_context(
                tc.tile_pool(name="pscore", bufs=2, space="PSUM"))
            pacc = ctx.enter_context(
                tc.tile_pool(name="pacc", bufs=2, space="PSUM"))
            pproj = ctx.enter_context(
                tc.tile_pool(name="pproj", bufs=2, space="PSUM"))
            dram = ctx.enter_context(
                tc.tile_pool(name="dram", bufs=1, space="DRAM"))

            # ---- input preloads first (gpsimd queue), then weights,
            # so LN stats start immediately and the sync queue only holds
            # xbar transposes ----
            lnins = {}
            for (nm_, src_, n2_) in (("x", x_d, 4), ("k", kin_d, 4),
                                     ("v", vin_d, 1)):
                for i2 in range(n2_):
                    xt = lnin.tile([128, 2, D], BF16, tag="lnin",
                                   name=f"ln_{nm_}{i2}")
                    eng = nc.sync if nm_ == "x" else nc.gpsimd
                    eng.dma_start(
                        xt[:],
                        src_[i2 * 256:(i2 + 1) * 256, :].rearrange(
                            "(a p) d -> p a d", a=2))
                    lnins[(nm_, i2)] = xt
            w_sb = {}
            for n in ("wq", "wk", "wv", "wo", "wp"):
                w_sb[n] = persist.tile([128, 4, D], BF16, tag=f"w_{n}",
                                       name=f"w_{n}")
                nc.gpsimd.dma_start(
                    w_sb[n][:], w_d[n].rearrange("(b p) j -> p b j", p=128))
            bq_sb = persist.tile([128, 4], F32, tag="bq")
            nc.sync.dma_start(bq_sb[:], bq_d.rearrange("(b p) -> p b", p=128))
            bk_sb = persist.tile([128, 4], F32, tag="bk")
            nc.sync.dma_start(bk_sb[:], bk_d.rearrange("(b p) -> p b", p=128))
            bv_sb = persist.tile([128, 4], F32, tag="bv")
            nc.sync.dma_start(bv_sb[:], bv_d.rearrange("(b p) -> p b", p=128))
            bo_row = persist.tile([1, D], BF16, tag="bo_row")
            nc.sync.dma_start(bo_row[:], bo_d.rearrange("(o d) -> o d", o=1))
            bp_row = persist.tile([1, D], BF16, tag="bp_row")
            nc.sync.dma_start(bp_row[:], bp_d.rearrange("(o d) -> o d", o=1))
            ones_row = persist.tile([1, 128], BF16, tag="ones_row")
            nc.vector.memset(ones_row[:], 1.0)
            # head-indicator for broadcasting the pair's two softmax
            # denominators across the 2x64 output rows: ind[hh, p]=1 iff
            # p//64 == hh  (same for every pair)
            ind = persist.tile([2, 128], BF16, tag="ind")
            nc.sync.dma_start(ind[:], ind_d)
            eye = persist.tile([128, 128], BF16, tag="eye")
            nc.sync.dma_start(eye[:], eye_d)
            eps_col = persist.tile([128, 1], F32, tag="eps_col")
            nc.vector.memset(eps_col[:], EPS)

            # ---- LN helper: stats on V, normalize on ACT, then XBAR
            # transpose into dstT[:, i, :, :] ----
            def ln_block(x_half, xn_half):
                # stats + normalize on V (one cross-engine hop: the sqrt)
                stats6 = small.tile([128, 6], F32, tag="bns")
                nc.vector.bn_stats(stats6[:], x_half)
                mv = small.tile([128, 2], F32, tag="bna")
                nc.vector.bn_aggr(mv[:], stats6[:])
                std = small.tile([128, 1], F32, tag="std")
                nc.scalar.activation(std[:], mv[:, 1:2], Sqrt,
                                     bias=eps_col[:, 0:1])
                rstd = small.tile([128, 1], F32, tag="rstd")
                nc.vector.reciprocal(rstd[:], std[:])
                nc.vector.tensor_scalar(
                    xn_half, x_half, mv[:, 0:1], rstd[:],
                    op0=AluOpType.subtract, op1=AluOpType.mult)

            def ln_batch(nm, n2tiles, xn, coff):
                # Batched stats on V (one sqrt hop), normalizes on the
                # otherwise-idle ACT via Identity with per-partition affine
                nh = n2tiles * 2
                xts = []
                mvs = []
                veps = small.tile([128, nh], F32, tag="vepsb",
                                  name=f"veps_{nm}")
                for i2 in range(n2tiles):
                    xt = lnins[(nm, i2)]
                    xts.append(xt)
                    for a in range(2):
                        h = i2 * 2 + a
                        stats6 = small.tile([128, 6], F32, tag="bns")
                        nc.vector.bn_stats(stats6[:], xt[:, a, :])
                        mv = small.tile([128, 2], F32, tag="bna",
                                        name=f"mv{h}")
                        nc.vector.bn_aggr(mv[:], stats6[:])
                        mvs.append(mv)
                        nc.vector.tensor_scalar_add(
                            veps[:, h:h + 1], mv[:, 1:2], EPS)
                std = small.tile([128, nh], F32, tag="stdb",
                                 name=f"std_{nm}")
                nc.scalar.sqrt(std[:], veps[:])
                rstd = small.tile([128, nh], F32, tag="rstdb",
                                  name=f"rstd_{nm}")
                nc.vector.reciprocal(rstd[:], std[:])
                for i2 in range(n2tiles):
                    for a in range(2):
                        h = i2 * 2 + a
                        if nm == "x":
                            # x normalizes on ACT, k/v on V: the two norm
                            # phases run in parallel after the stats
                            nmr = small.tile([128, 1], F32, tag="nmr",
                                             name=f"nmr{h}")
                            nc.vector.tensor_scalar(
                                nmr[:], mvs[h][:, 0:1], rstd[:, h:h + 1],
                                -1.0,
                                op0=AluOpType.mult, op1=AluOpType.mult)
                            nc.scalar.activation(
                                xn[:, coff + h, :], xts[i2][:, a, :], Ident,
                                bias=nmr[:, 0:1], scale=rstd[:, h:h + 1])
                        else:
                            nc.vector.tensor_scalar(
                                xn[:, coff + h, :], xts[i2][:, a, :],
                                mvs[h][:, 0:1], rstd[:, h:h + 1],
                                op0=AluOpType.subtract, op1=AluOpType.mult)

            qT = persist.tile([128, 4, BT], BF16, tag="qT")
            kT = persist.tile([128, 4, CUS], BF16, tag="kT")

            def project(dstT, wname, srcT, bias_sb, jt):
                for tch in range(2):
                    ps = pproj.tile([128, 512], F32, tag="pj")
                    for dt in range(4):
                        nc.tensor.matmul(
                            ps[:],
                            lhsT=w_sb[wname][:, dt, jt * 128:(jt + 1) * 128],
                            rhs=srcT[:, tch * 4:(tch + 1) * 4, dt, :],
                            start=(dt == 0), stop=(dt == 3))
                    nc.vector.tensor_scalar_add(
                        dstT[:, jt, tch * 512:(tch + 1) * 512], ps[:],
                        bias_sb[:, jt:jt + 1])

            xknT = persist.tile([128, 16, 4, 128], BF16, tag="xknT")
            xnT = xknT[:, 0:8, :, :]
            knT = xknT[:, 8:16, :, :]
            xkn = lnout.tile([128, 16, D], BF16, tag="lnout", name="xkn")
            ln_batch("x", 4, xkn, 0)
            ln_batch("k", 4, xkn, 8)
            # ONE xbar transpose for x+k together (each transpose carries a
            # ~10.4us completion-semaphore latency, so fewer is faster):
            # transposed row r = h*512 + j*128 + p lands on (h, j, p)
            nc.sync.dma_start(
                xknT[:], xkn.rearrange("p h d -> p (h d)"), transpose=True)
            project(qT, "wq", xnT, bq_sb, 0)
            project(kT, "wk", knT, bk_sb, 0)
            vnT = persist.tile([128, 2, 4, 128], BF16, tag="vnT")
            vn = lnout.tile([128, 2, D], BF16, tag="lnv", name="vn")
            ln_batch("v", 1, vn, 0)
            nc.sync.dma_start(
                vnT[:], vn.rearrange("p h d -> p (h d)"), transpose=True)


            # ---- v projection (+ ones column for the denominator) ----
            v_sb = []
            for ct in range(2):
                vt = persist.tile([128, 8, 65], BF16, tag=f"v_sb{ct}")
                ps = pproj.tile([128, 512], F32, tag="pj")
                for dt in range(4):
                    nc.tensor.matmul(
                        ps[:], lhsT=vnT[:, ct, dt, :],
                        rhs=w_sb["wv"][:, dt, :],
                        start=(dt == 0), stop=(dt == 3))
                nc.vector.tensor_copy(
                    vt[:, :, 0:64],
                    ps[:].rearrange("p (h e) -> p h e", h=8))
                nc.vector.memset(vt[:, :, 64:65], 1.0)
                v_sb.append(vt)

            # ---- bounce buffers: asymmetric RS groups {0}, {1,2}, {3}:
            # the premium-paying first collective fires right after pair 0;
            # {1,2} starts the moment it ends; only pair 3's small op
            # remains on the critical path ----
            bounce_ins = [
                dram.tile([NCORES, 130, TSH], BF16, name=f"bin{i}")
                for i in range(4)
            ]
            bounce_outs = [
                dram.tile([130, TSH], BF16, name=f"bout{i}")
                for i in range(4)
            ]
            bviews = [
                bo_.rearrange("(q h j) t -> q h j t", q=1, h=2)
                for bo_ in bounce_outs
            ]
            RS_G = {0: (0, 0), 1: (1, 0), 2: (2, 0), 3: (3, 0)}
            ob = persist.tile([128, 4, TSH], BF16, tag="ob")
            o_n = persist.tile([128, 4, TSH], BF16, tag="o_n")
            s_bfs = []

            def emit_attnv(ajt, apm):
                # attn @ v_aug for pair ajt (pso [65, 512] per (head, tch))
                o65s = []
                for hh in range(2):
                    o65 = o65p.tile([65, 1024], BF16, tag="o65")
                    for tch in range(2):
                        pso = pacc.tile([65, 512], F32, tag="av")
                        for c2 in range(2):
                            nc.tensor.matmul(
                                pso[:],
                                lhsT=v_sb[c2][:, 2 * ajt + hh, :],
                                rhs=apm[(hh, c2)][:,
                                                  tch * 512:(tch + 1) * 512],
                                start=(c2 == 0), stop=(c2 == 1))
                        if tch == 0:
                            nc.scalar.copy(o65[:, 0:512], pso[:])
                        else:
                            nc.vector.tensor_copy(o65[:, 512:1024], pso[:])
                    o65s.append(o65)

                g, qoff = RS_G[ajt]
                b_in = bounce_ins[g]
                for hh in range(2):
                    nc.sync.dma_start(
                        b_in[:, qoff + hh * 65:
                             qoff + (hh + 1) * 65, :].rearrange(
                            "s r t -> r s t"),
                        o65s[hh].rearrange("r (s t) -> r s t", s=NCORES))
                if True:
                    nc.gpsimd.collective_compute(
                        "ReduceScatter", AluOpType.add,
                        replica_groups=[list(range(NCORES))],
                        ins=[b_in.rearrange("s r t -> (s r) t")],
                        outs=[bounce_outs[g].opt()],
                    )

            pending = []
            for jt in range(4):
                # scores: r-tile covers cu rows r*128..r*128+127
                # (u = r//2, c2 = r%2); head 2jt on partitions 0:64 (tile
                # row 0), head 2jt+1 on 64:128 (tile row 64) -> concurrent.
                cA = [None, None]  # exp-first chains (bf16), head 2jt
                mB = [None, None]  # max-first chains (fp32), head 2jt+1
                pm = {}
                for r in range(8):
                    c2 = r % 2
                    pa = pscore.tile([128, 1024], F32, tag="sc")
                    pb = pscore.tile([128, 1024], F32, tag="sc")
                    for tch in range(2):
                        nc.tensor.matmul(
                            pa[:, tch * 512:(tch + 1) * 512],
                            lhsT=kT[0:64, jt, r * 128:(r + 1) * 128],
                            rhs=qT[0:64, jt, tch * 512:(tch + 1) * 512],
                            start=True, stop=True)
                        nc.tensor.matmul(
                            pb[:, tch * 512:(tch + 1) * 512],
                            lhsT=kT[64:128, jt, r * 128:(r + 1) * 128],
                            rhs=qT[64:128, jt, tch * 512:(tch + 1) * 512],
                            start=True, stop=True)
                    # head 2jt: exp-first (ACT exp per tile), bf16 max
                    # chain on V at 2x
                    if r < 2:
                        et = pmpool.tile([128, 1024], BF16, tag="ca")
                        nc.scalar.activation(et[:], pa[:], Exp)
                        cA[c2] = et
                    else:
                        et = epool.tile([128, 1024], BF16, tag="e")
                        nc.scalar.activation(et[:], pa[:], Exp)
                        if r >= 6:
                            dst = pmpool.tile([128, 1024], BF16, tag="pm",
                                              name="pmA")
                        else:
                            dst = pmpool.tile([128, 1024], BF16, tag="ca",
                                              name="caA")
                        nc.vector.tensor_max(dst[:], cA[c2][:], et[:])
                        cA[c2] = dst
                    # head 2jt+1: max-first chain (seed on ACT, maxes on V)
                    if r < 2:
                        m = mpool.tile([128, 1024], F32, tag="macc")
                        nc.scalar.copy(m[:], pb[:])
                        mB[c2] = m
                    else:
                        m = mpool.tile([128, 1024], F32, tag="macc")
                        nc.vector.tensor_max(m[:], mB[c2][:], pb[:])
                        mB[c2] = m
                    # previous pair's attnv + bounce keeps the PE warm
                    # across the pair boundary
                    if r == 1 and pending:
                        emit_attnv(*pending.pop())
                    # next pair's projections fill the PE's drain-shadow
                    if r == 4 and jt < 3:
                        project(qT, "wq", xnT, bq_sb, jt + 1)
                        project(kT, "wk", knT, bk_sb, jt + 1)

                for c2 in range(2):
                    pm[(0, c2)] = cA[c2]
                    pb = pmpool.tile([128, 1024], BF16, tag="pm")
                    nc.scalar.activation(pb[:], mB[c2][:], Exp)
                    pm[(1, c2)] = pb

                pending.append((jt, pm))
                if jt == 0:
                    emit_attnv(*pending.pop())
            emit_attnv(*pending.pop())

            # ---- readbacks after all RS triggers ----
            for pj in range(4):
                g, qoff = RS_G[pj]
                bv_ = bviews[g][qoff // 130]
                for hh in range(2):
                    nc.gpsimd.dma_start(
                        ob[hh * 64:(hh + 1) * 64, pj, :],
                        bv_[hh, 0:64, :])
                s_bf = small.tile([2, TSH], BF16, tag="sbf",
                                  name=f"sbf{pj}")
                nc.gpsimd.dma_start(s_bf[:], bv_[:, 64, :])
                s_bfs.append(s_bf)

            # ---- deferred per-pair divide + wo partials, grouped so
            # group 0's divides and wo matmuls run during RS-B ----
            psy = pproj.tile([128, 512], F32, tag="pj")
            for pj in range(4):
                psden = pscore.tile([128, TSH], F32, tag="sc", name="psden")
                nc.tensor.matmul(psden[:], lhsT=ind[:], rhs=s_bfs[pj][:],
                                 start=True, stop=True)
                rb = small.tile([128, TSH], F32, tag="rb")
                nc.vector.reciprocal(rb[:], psden[:])
                tmp = small.tile([128, TSH], F32, tag="odiv")
                nc.vector.tensor_mul(tmp[:], ob[:, pj, :], rb[:])
                nc.vector.tensor_scalar_add(
                    o_n[:, pj, :], tmp[:], bv_sb[:, pj:pj + 1])
                nc.tensor.matmul(psy[:], lhsT=o_n[:, pj, :],
                                 rhs=w_sb["wo"][:, pj, :],
                                 start=(pj == 0), stop=False)

            # ---- final: + bo, LN4 (PE transposes: the xbar's ~10.4us
            # completion latency is too slow for the tail), wp + bp ----
            nc.tensor.matmul(psy[:], lhsT=ones_row[:],
                             rhs=bo_row[:], start=False, stop=True)
            # LN4 stats straight from PSUM (no y1 staging copy); chunked
            # normalize so the zT transposes and wp matmuls pipeline
            stats6 = small.tile([128, 6], F32, tag="bns")
            nc.vector.bn_stats(stats6[:], psy[:])
            mv4 = small.tile([128, 2], F32, tag="bna", name="mv4")
            nc.vector.bn_aggr(mv4[:], stats6[:])
            std4 = small.tile([128, 1], F32, tag="std", name="std4")
            nc.scalar.activation(std4[:], mv4[:, 1:2], Sqrt,
                                 bias=eps_col[:, 0:1])
            rstd4 = small.tile([128, 1], F32, tag="rstd", name="rstd4")
            nc.vector.reciprocal(rstd4[:], std4[:])
            z = persist.tile([128, D], BF16, tag="z")
            zT = persist.tile([128, 4, 128], BF16, tag="zT")
            psy2 = pproj.tile([128, 512], F32, tag="pj")
            for b in range(4):
                nc.vector.tensor_scalar(
                    z[:, b * 128:(b + 1) * 128],
                    psy[:, b * 128:(b + 1) * 128], mv4[:, 0:1], rstd4[:],
                    op0=AluOpType.subtract, op1=AluOpType.mult)
                tp = pacc.tile([128, 128], BF16, tag="av", name="tpz")
                nc.tensor.transpose(tp[:], z[:, b * 128:(b + 1) * 128],
                                    eye[:])
                nc.scalar.copy(zT[:, b, :], tp[:])
                nc.tensor.matmul(psy2[:], lhsT=zT[:, b, :],
                                 rhs=w_sb["wp"][:, b, :],
                                 start=(b == 0), stop=False)
            nc.tensor.matmul(psy2[:], lhsT=ones_row[:],
                             rhs=bp_row[:], start=False, stop=True)
            yt = persist.tile([128, P], F32, tag="yt")
            nc.vector.tensor_copy(yt[:], psy2[:])
            nc.sync.dma_start(y_d[:], yt[:])

    nc.compile()
    return nc


def _prep_host(inputs):
    """Fold LN weights/biases and 1/sqrt(dk) into projection weights; build
    per-core input maps."""
    f32 = np.float32
    bf16 = ml_dtypes.bfloat16
    me = np.ascontiguousarray(
        inputs["model_embed"], dtype=f32).reshape(BT, D).astype(bf16)
    kin = np.asarray(inputs["context_embed_key"], dtype=f32)
    vin = np.asarray(inputs["context_embed_value"], dtype=f32)
    g = lambda n: np.asarray(inputs[n], dtype=f32)

    scale = 1.0 / np.sqrt(DK)
    wq_eff = (g("ln1_w")[:, None] * g("wq")) * scale
    bq_eff = (g("ln1_b") @ g("wq") + g("bq")) * scale
    wk_eff = g("ln2_w")[:, None] * g("wk")
    bk_eff = g("ln2_b") @ g("wk") + g("bk")
    wv_eff = g("ln3_w")[:, None] * g("wv")
    bv_eff = g("ln3_b") @ g("wv") + g("bv")
    wo_eff = g("wo")
    bo_eff = g("bo")
    wp_eff = g("ln4_w")[:, None] * g("wp")
    bp_eff = g("ln4_b") @ g("wp") + g("bp")

    # overflow guard for the zero-offset softmax: |logits| must stay << 87
    def smax(w):
        v = np.random.RandomState(0).randn(w.shape[1]).astype(f32)
        for _ in range(20):
            v = w.T @ (w @ v)
            v /= np.linalg.norm(v)
        return np.linalg.norm(w @ v)
    bound = ((np.sqrt(D) * smax(wq_eff) + np.linalg.norm(bq_eff))
             * (np.sqrt(D) * smax(wk_eff) + np.linalg.norm(bk_eff)))
    assert bound < 80.0, f"logit bound {bound} too large for exp without max"

    ind = np.zeros((2, 128), np.float32)
    ind[0, 0:64] = 1.0
    ind[1, 64:128] = 1.0

    common = {
        "x": me,
        "wq": wq_eff.astype(bf16), "wk": wk_eff.astype(bf16),
        "wv": wv_eff.astype(bf16), "wo": wo_eff.astype(bf16),
        "wp": wp_eff.astype(bf16),
        "bq": bq_eff, "bk": bk_eff, "bv": bv_eff,
        "bo": bo_eff.astype(bf16), "bp": bp_eff.astype(bf16),
        "ind": ind.astype(bf16),
        "eye": np.eye(128, dtype=bf16),
    }
    in_maps = []
    for c in range(NCORES):
        ksh = kin[c * CS:(c + 1) * CS]             # [CS, U, D]
        ksh = np.ascontiguousarray(
            ksh.transpose(1, 0, 2).reshape(CUS, D)).astype(bf16)
        vsh = np.ascontiguousarray(vin[c * CS:(c + 1) * CS]).astype(bf16)
        m = dict(common)
        m["kin"] = ksh
        m["vin"] = vsh
        in_maps.append(m)
    return in_maps


def kernel(**inputs) -> np.ndarray:
    if "nc" not in _CACHE:
        _CACHE["nc"] = build_nc()
    nc = _CACHE["nc"]
    in_maps = _prep_host(inputs)
    res = bass_utils.run_bass_kernel_spmd(
        nc, in_maps, core_ids=list(range(NCORES)))
    y = np.concatenate([res.results[c]["y"] for c in range(NCORES)], axis=0)
    return y.reshape(B, T, P).astype(np.float32)


if __name__ == "__main__":
    print("building...")
    build_nc()
    print("ok")
